# revision 3
# baseline (speedup 1.0000x reference)
"""DotLinkPredictor kernel v2: 4 SWDGE queues + deeper gather pipeline.

score[e] = dot(h[src[e]], h[dst[e]]),  E = 1,000,000 edges, h: [100000, 64] f32.

Same architecture as baseline (edges sharded 8 ways, h replicated, 16
(src-chunk, dst-chunk) segments with int16 chunk-local dma_gather), plus:
  - num_swdge_queues=4; consecutive gathers cycle queues 0..3 so all four
    SDMA queue rings drain concurrently (measured 2.6x vs 2 queues).
  - 5 segments in flight (5 hu/hv buffer pairs) to keep the queues fed.
  - num_idxs_reg matches num_idxs (no per-window reg_load / cnt input);
    window tails are 0-padded (pads gather node 0 into unread slots), so the
    Q7 trim count always equals the NX ring reservation.
"""

import numpy as np

import concourse.bacc as bacc
import concourse.mybir as mybir
from concourse.bass_utils import run_bass_kernel_spmd

N_NODES = 100000
D = 64
N_EDGES = 1000000
N_CORES = 8
EPC = N_EDGES // N_CORES      # 125000 edges per core
P = 128
CHUNK = 25000                 # 4 node chunks, int16-addressable
NSEG = 16                     # (src_chunk, dst_chunk) pairs
CAP_MIN = 1024                # per-segment capacity floor (multiple of 128)
MAX_SUB = 1024                # max num_idxs per dma_gather on this target
N_QUEUES = 4
DEPTH = 5                     # segments in flight

_PROG_CACHE: dict[tuple, object] = {}


def _subs_for(cap):
    subs = [MAX_SUB] * (cap // MAX_SUB)
    if cap % MAX_SUB:
        subs.append(cap % MAX_SUB)
    return subs


def _build(cap, reps=1, nq=N_QUEUES, depth=DEPTH, mode="full", sp=True,
           scratch=16384):
    """One SPMD program; each core runs it on its own segment-grouped shard.

    reps>1 replays the whole pipeline (same data) for benchmarking: device
    time scales by reps while RPC overhead doesn't.  mode: "full" |
    "gather" (no DVE math) | "compute" (no gathers) — bench-only knobs.
    """
    assert cap % P == 0
    idxc = cap // 16          # idx columns per segment (16-wrapped)
    colseg = cap // P         # score columns per segment
    subs = _subs_for(cap)

    nc = bacc.Bacc("TRN2", target_bir_lowering=False, debug=False,
                   num_swdge_queues=nq, dynamic_dma_scratch_size=scratch)
    h_t = nc.dram_tensor("h", [N_NODES, D], mybir.dt.float32,
                         kind="ExternalInput")
    su_t = nc.dram_tensor("su", [P, NSEG * idxc], mybir.dt.int16,
                          kind="ExternalInput")
    sv_t = nc.dram_tensor("sv", [P, NSEG * idxc], mybir.dt.int16,
                          kind="ExternalInput")
    out_t = nc.dram_tensor("scores", [P, NSEG * colseg], mybir.dt.float32,
                           kind="ExternalOutput")

    su_s = nc.alloc_sbuf_tensor("su_s", [P, NSEG * idxc], mybir.dt.int16)
    sv_s = nc.alloc_sbuf_tensor("sv_s", [P, NSEG * idxc], mybir.dt.int16)
    hu = [nc.alloc_sbuf_tensor(f"hu{i}", [P, colseg, D], mybir.dt.float32)
          for i in range(depth)]
    hv = [nc.alloc_sbuf_tensor(f"hv{i}", [P, colseg, D], mybir.dt.float32)
          for i in range(depth)]
    sc = nc.alloc_sbuf_tensor("sc", [P, NSEG * colseg], mybir.dt.float32)

    gathers_per_seg = len(subs)  # per endpoint

    import contextlib
    with contextlib.ExitStack() as stack:
        block = stack.enter_context(nc.Block())
        out_sem = stack.enter_context(nc.semaphore("out_sem"))
        gu_sems = [stack.enter_context(nc.semaphore(f"gu{s}"))
                   for s in range(NSEG)]
        gv_sems = [stack.enter_context(nc.semaphore(f"gv{s}"))
                   for s in range(NSEG)]
        comp_sems = [stack.enter_context(nc.semaphore(f"cmp{s}"))
                     for s in range(NSEG)]
        mul_sems = [stack.enter_context(nc.semaphore(f"mul{s}"))
                    for s in range(NSEG)]
        zs_sem = stack.enter_context(nc.semaphore("zs_sem"))
        idxp_sems = [stack.enter_context(nc.semaphore(f"ixp{s}"))
                     for s in range(NSEG)]

        @block.sync
        def _(s):
            # per-segment idx slices so segment 0's gathers can start while
            # later segments' indices still stream in
            for i in range(NSEG):
                cols = slice(i * idxc, (i + 1) * idxc)
                s.dma_start(out=su_s[:, cols], in_=su_t[:, cols]
                            ).then_inc(idxp_sems[i], 16)
                s.dma_start(out=sv_s[:, cols], in_=sv_t[:, cols]
                            ).then_inc(idxp_sems[i], 16)
            if mode == "gfree":
                # free-running gather benchmark: no stores, just completion
                for i in range(NSEG):
                    s.wait_ge(gu_sems[i], 16 * len(subs) * reps)
                    s.wait_ge(gv_sems[i], 16 * len(subs) * reps)
                s.dma_start(out=out_t[:, 0:colseg], in_=sc[:, 0:colseg]
                            ).then_inc(out_sem, 16)
                s.wait_ge(out_sem, 16)
                return
            # store each segment's scores as soon as its compute finishes,
            # overlapping stores with the remaining segments' gathers
            for r in range(reps):
                for i in range(NSEG):
                    sl = slice(i * colseg, (i + 1) * colseg)
                    s.wait_ge(comp_sems[i], r + 1)
                    s.dma_start(out=out_t[:, sl], in_=sc[:, sl]
                                ).then_inc(out_sem, 16)
            s.wait_ge(out_sem, 16 * NSEG * reps)

        @block.gpsimd
        def _(g):
            if mode == "compute":
                return
            qn = 0
            for r in range(reps):
                for s in range(NSEG):
                    cu, cv = s // 4, s % 4
                    t = r * NSEG + s
                    if r == 0:
                        g.wait_ge(idxp_sems[s], 32)
                    if t >= depth and mode != "gfree":
                        pr, ps = divmod(t - depth, NSEG)
                        g.wait_ge(comp_sems[ps], pr + 1)
                    hub, hvb = hu[t % depth], hv[t % depth]
                    in_u = h_t[cu * CHUNK:(cu + 1) * CHUNK, :]
                    in_v = h_t[cv * CHUNK:(cv + 1) * CHUNK, :]
                    off = 0
                    for gi, nidx in enumerate(subs):
                        c0 = s * idxc + off // 16
                        ccols = nidx // 16
                        o0 = off // P
                        ocols = nidx // P
                        g.dma_gather(
                            out_ap=hub[:, o0:o0 + ocols, :], in_ap=in_u,
                            idxs_ap=su_s[:, c0:c0 + ccols],
                            num_idxs=nidx, num_idxs_reg=nidx, elem_size=D,
                            queue_num=qn % nq, single_packet=sp,
                        ).then_inc(gu_sems[s], 16)
                        g.dma_gather(
                            out_ap=hvb[:, o0:o0 + ocols, :], in_ap=in_v,
                            idxs_ap=sv_s[:, c0:c0 + ccols],
                            num_idxs=nidx, num_idxs_reg=nidx, elem_size=D,
                            queue_num=(qn + 1) % nq, single_packet=sp,
                        ).then_inc(gv_sems[s], 16)
                        qn += 2
                        off += nidx

        @block.vector
        def _(v):
            if mode == "gfree":
                return
            for r in range(reps):
                for s in range(NSEG):
                    t = r * NSEG + s
                    if mode != "compute":
                        v.wait_ge(gu_sems[s], 16 * gathers_per_seg * (r + 1))
                        v.wait_ge(gv_sems[s], 16 * gathers_per_seg * (r + 1))
                    if r > 0:
                        # rep r-1's store of this segment must have left
                        v.wait_ge(out_sem, 16 * ((r - 1) * NSEG + s + 1))
                    hub, hvb = hu[t % depth], hv[t % depth]
                    if mode == "gather":
                        v.memset(sc[:, s * colseg:s * colseg + 1], 0.0
                                 ).then_inc(comp_sems[s], 1)
                        continue
                    v.tensor_tensor(out=hub[:], in0=hub[:], in1=hvb[:],
                                    op=mybir.AluOpType.mult
                                    ).then_inc(mul_sems[s], 1)
                    v.wait_ge(mul_sems[s], r + 1)
                    v.tensor_reduce(
                        out=sc[:, s * colseg:(s + 1) * colseg], in_=hub[:],
                        axis=mybir.AxisListType.X, op=mybir.AluOpType.add,
                    ).then_inc(comp_sems[s], 1)

    nc.compile()
    return nc


def _get_prog(cap):
    if cap not in _PROG_CACHE:
        _PROG_CACHE[cap] = _build(cap)
    return _PROG_CACHE[cap]


def _wrap16(padded, cap):
    """[NSEG, cap] -> [128, NSEG * cap//16] in dma_gather's wrapped layout."""
    idxc = cap // 16
    w = padded.reshape(NSEG, idxc, 16).transpose(0, 2, 1)      # [s, 16, idxc]
    w = np.tile(w, (1, P // 16, 1))                            # [s, 128, idxc]
    return np.ascontiguousarray(
        w.transpose(1, 0, 2).reshape(P, NSEG * idxc)
    )


def _prepare(h, src, dst):
    """Host prep: shard, segment-sort, 0-pad window tails (pads gather node 0)."""
    h = np.ascontiguousarray(np.asarray(h), dtype=np.float32)
    src = np.asarray(src).astype(np.int32)
    dst = np.asarray(dst).astype(np.int32)

    in_maps = []
    recon = []   # (order, p, col) per core
    cap = CAP_MIN
    shard_data = []
    for c in range(N_CORES):
        s_loc = src[c * EPC:(c + 1) * EPC]
        d_loc = dst[c * EPC:(c + 1) * EPC]
        cu = s_loc // CHUNK
        cv = d_loc // CHUNK
        key = (cu * 4 + cv).astype(np.uint8)
        order = np.argsort(key, kind="stable")
        key_sorted = key[order]
        counts = np.bincount(key, minlength=NSEG)
        cap = max(cap, int(np.ceil(counts.max() / P)) * P)
        shard_data.append((s_loc, d_loc, cu, cv, key, order, key_sorted, counts))

    colseg = cap // P
    for c in range(N_CORES):
        s_loc, d_loc, cu, cv, key, order, key_sorted, counts = shard_data[c]
        starts = np.zeros(NSEG, dtype=np.int64)
        starts[1:] = np.cumsum(counts)[:-1]
        r_in_seg = np.arange(EPC, dtype=np.int64) - starts[key_sorted]

        lu = (s_loc - cu * CHUNK).astype(np.int16)
        lv = (d_loc - cv * CHUNK).astype(np.int16)
        pu = np.zeros((NSEG, cap), dtype=np.int16)
        pv = np.zeros((NSEG, cap), dtype=np.int16)
        pu[key_sorted, r_in_seg] = lu[order]
        pv[key_sorted, r_in_seg] = lv[order]

        in_maps.append({
            "h": h,
            "su": _wrap16(pu, cap),
            "sv": _wrap16(pv, cap),
        })
        p = (r_in_seg % P).astype(np.int64)
        col = (key_sorted.astype(np.int64) * colseg
               + 8 * (r_in_seg // MAX_SUB) + (r_in_seg % MAX_SUB) // P)
        recon.append((order, p, col))
    return in_maps, recon, cap


def kernel(h, src, dst):
    in_maps, recon, cap = _prepare(h, src, dst)
    nc = _get_prog(cap)
    res = run_bass_kernel_spmd(nc, in_maps, list(range(N_CORES)))

    out = np.empty(N_EDGES, dtype=np.float32)
    for c in range(N_CORES):
        order, p, col = recon[c]
        scores = res.results[c]["scores"]          # [128, NSEG*colseg]
        shard = np.empty(EPC, dtype=np.float32)
        shard[order] = scores[p, col]
        out[c * EPC:(c + 1) * EPC] = shard
    return out


# revision 5
# speedup vs baseline: 1.2076x; 1.2076x over previous
"""DotLinkPredictor kernel v2: 4 SWDGE queues + deeper gather pipeline.

score[e] = dot(h[src[e]], h[dst[e]]),  E = 1,000,000 edges, h: [100000, 64] f32.

Same architecture as baseline (edges sharded 8 ways, h replicated, 16
(src-chunk, dst-chunk) segments with int16 chunk-local dma_gather), plus:
  - num_swdge_queues=4; consecutive gathers cycle queues 0..3 so all four
    SDMA queue rings drain concurrently (measured 2.6x vs 2 queues).
  - 5 segments in flight (5 hu/hv buffer pairs) to keep the queues fed.
  - num_idxs_reg matches num_idxs (no per-window reg_load / cnt input);
    window tails are 0-padded (pads gather node 0 into unread slots), so the
    Q7 trim count always equals the NX ring reservation.
"""

import numpy as np

import concourse.bacc as bacc
import concourse.mybir as mybir
from concourse.bass_utils import run_bass_kernel_spmd

N_NODES = 100000
D = 64
N_EDGES = 1000000
N_CORES = 8
EPC = N_EDGES // N_CORES      # 125000 edges per core
P = 128
CHUNK = 25000                 # 4 node chunks, int16-addressable
NSEG = 16                     # (src_chunk, dst_chunk) pairs
CAP_MIN = 1024                # per-segment capacity floor (multiple of 128)
MAX_SUB = 1024                # max num_idxs per dma_gather on this target
N_QUEUES = 4
DEPTH = 5                     # segments in flight

_PROG_CACHE: dict[tuple, object] = {}


def _subs_for(cap):
    subs = [MAX_SUB] * (cap // MAX_SUB)
    if cap % MAX_SUB:
        subs.append(cap % MAX_SUB)
    return subs


def _build(cap, reps=1, nq=N_QUEUES, depth=DEPTH, mode="full", sp=True,
           scratch=16384):
    """One SPMD program; each core runs it on its own segment-grouped shard.

    reps>1 replays the whole pipeline (same data) for benchmarking: device
    time scales by reps while RPC overhead doesn't.  mode: "full" |
    "gather" (no DVE math) | "compute" (no gathers) — bench-only knobs.
    """
    assert cap % P == 0
    idxc = cap // 16          # idx columns per segment (16-wrapped)
    colseg = cap // P         # score columns per segment
    subs = _subs_for(cap)

    nc = bacc.Bacc("TRN2", target_bir_lowering=False, debug=False,
                   num_swdge_queues=nq, dynamic_dma_scratch_size=scratch)
    h_t = nc.dram_tensor("h", [N_NODES, D], mybir.dt.float32,
                         kind="ExternalInput")
    su_t = nc.dram_tensor("su", [P, NSEG * idxc], mybir.dt.int16,
                          kind="ExternalInput")
    sv_t = nc.dram_tensor("sv", [P, NSEG * idxc], mybir.dt.int16,
                          kind="ExternalInput")
    out_t = nc.dram_tensor("scores", [P, NSEG * colseg], mybir.dt.float32,
                           kind="ExternalOutput")

    su_s = nc.alloc_sbuf_tensor("su_s", [P, NSEG * idxc], mybir.dt.int16)
    sv_s = nc.alloc_sbuf_tensor("sv_s", [P, NSEG * idxc], mybir.dt.int16)
    hu = [nc.alloc_sbuf_tensor(f"hu{i}", [P, colseg, D], mybir.dt.float32)
          for i in range(depth)]
    hv = [nc.alloc_sbuf_tensor(f"hv{i}", [P, colseg, D], mybir.dt.float32)
          for i in range(depth)]
    sc = nc.alloc_sbuf_tensor("sc", [P, NSEG * colseg], mybir.dt.float32)

    gathers_per_seg = len(subs)  # per endpoint

    import contextlib
    with contextlib.ExitStack() as stack:
        block = stack.enter_context(nc.Block())
        out_sem = stack.enter_context(nc.semaphore("out_sem"))
        gu_sems = [stack.enter_context(nc.semaphore(f"gu{s}"))
                   for s in range(NSEG)]
        gv_sems = [stack.enter_context(nc.semaphore(f"gv{s}"))
                   for s in range(NSEG)]
        comp_sems = [stack.enter_context(nc.semaphore(f"cmp{s}"))
                     for s in range(NSEG)]
        zs_sem = stack.enter_context(nc.semaphore("zs_sem"))
        idxp_sems = [stack.enter_context(nc.semaphore(f"ixp{s}"))
                     for s in range(NSEG)]

        @block.sync
        def _(s):
            # per-segment idx slices so segment 0's gathers can start while
            # later segments' indices still stream in
            for i in range(NSEG):
                cols = slice(i * idxc, (i + 1) * idxc)
                s.dma_start(out=su_s[:, cols], in_=su_t[:, cols]
                            ).then_inc(idxp_sems[i], 16)
                s.dma_start(out=sv_s[:, cols], in_=sv_t[:, cols]
                            ).then_inc(idxp_sems[i], 16)
            if mode == "gfree":
                # free-running gather benchmark: no stores, just completion
                for i in range(NSEG):
                    s.wait_ge(gu_sems[i], 16 * len(subs) * reps)
                    s.wait_ge(gv_sems[i], 16 * len(subs) * reps)
                s.dma_start(out=out_t[:, 0:colseg], in_=sc[:, 0:colseg]
                            ).then_inc(out_sem, 16)
                s.wait_ge(out_sem, 16)
                return
            # store each segment's scores as soon as its compute finishes,
            # overlapping stores with the remaining segments' gathers
            for r in range(reps):
                for i in range(NSEG):
                    sl = slice(i * colseg, (i + 1) * colseg)
                    s.wait_ge(comp_sems[i], r + 1)
                    s.dma_start(out=out_t[:, sl], in_=sc[:, sl]
                                ).then_inc(out_sem, 16)
            s.wait_ge(out_sem, 16 * NSEG * reps)

        @block.gpsimd
        def _(g):
            if mode == "compute":
                return
            qn = 0
            for r in range(reps):
                for s in range(NSEG):
                    cu, cv = s // 4, s % 4
                    t = r * NSEG + s
                    if r == 0:
                        g.wait_ge(idxp_sems[s], 32)
                    if t >= depth and mode != "gfree":
                        pr, ps = divmod(t - depth, NSEG)
                        g.wait_ge(comp_sems[ps], pr + 1)
                    hub, hvb = hu[t % depth], hv[t % depth]
                    in_u = h_t[cu * CHUNK:(cu + 1) * CHUNK, :]
                    in_v = h_t[cv * CHUNK:(cv + 1) * CHUNK, :]
                    off = 0
                    for gi, nidx in enumerate(subs):
                        c0 = s * idxc + off // 16
                        ccols = nidx // 16
                        o0 = off // P
                        ocols = nidx // P
                        g.dma_gather(
                            out_ap=hub[:, o0:o0 + ocols, :], in_ap=in_u,
                            idxs_ap=su_s[:, c0:c0 + ccols],
                            num_idxs=nidx, num_idxs_reg=nidx, elem_size=D,
                            queue_num=qn % nq, single_packet=sp,
                        ).then_inc(gu_sems[s], 16)
                        g.dma_gather(
                            out_ap=hvb[:, o0:o0 + ocols, :], in_ap=in_v,
                            idxs_ap=sv_s[:, c0:c0 + ccols],
                            num_idxs=nidx, num_idxs_reg=nidx, elem_size=D,
                            queue_num=(qn + 1) % nq, single_packet=sp,
                        ).then_inc(gv_sems[s], 16)
                        qn += 2
                        off += nidx

        @block.vector
        def _(v):
            if mode == "gfree":
                return
            for r in range(reps):
                for s in range(NSEG):
                    t = r * NSEG + s
                    if mode != "compute":
                        v.wait_ge(gu_sems[s], 16 * gathers_per_seg * (r + 1))
                        v.wait_ge(gv_sems[s], 16 * gathers_per_seg * (r + 1))
                    if r > 0:
                        # rep r-1's store of this segment must have left
                        v.wait_ge(out_sem, 16 * ((r - 1) * NSEG + s + 1))
                    hub, hvb = hu[t % depth], hv[t % depth]
                    if mode == "gather":
                        v.memset(sc[:, s * colseg:s * colseg + 1], 0.0
                                 ).then_inc(comp_sems[s], 1)
                        continue
                    # DVE is in-order: reduce follows the mult without a sem
                    v.tensor_tensor(out=hub[:], in0=hub[:], in1=hvb[:],
                                    op=mybir.AluOpType.mult)
                    v.tensor_reduce(
                        out=sc[:, s * colseg:(s + 1) * colseg], in_=hub[:],
                        axis=mybir.AxisListType.X, op=mybir.AluOpType.add,
                    ).then_inc(comp_sems[s], 1)

    nc.compile()
    return nc


def _get_prog(cap):
    if cap not in _PROG_CACHE:
        _PROG_CACHE[cap] = _build(cap)
    return _PROG_CACHE[cap]


def _wrap16(padded, cap):
    """[NSEG, cap] -> [128, NSEG * cap//16] in dma_gather's wrapped layout."""
    idxc = cap // 16
    w = padded.reshape(NSEG, idxc, 16).transpose(0, 2, 1)      # [s, 16, idxc]
    w = np.tile(w, (1, P // 16, 1))                            # [s, 128, idxc]
    return np.ascontiguousarray(
        w.transpose(1, 0, 2).reshape(P, NSEG * idxc)
    )


def _prepare(h, src, dst):
    """Host prep: shard, segment-sort, 0-pad window tails (pads gather node 0)."""
    h = np.ascontiguousarray(np.asarray(h), dtype=np.float32)
    src = np.asarray(src).astype(np.int32)
    dst = np.asarray(dst).astype(np.int32)

    in_maps = []
    recon = []   # (order, p, col) per core
    cap = CAP_MIN
    shard_data = []
    for c in range(N_CORES):
        s_loc = src[c * EPC:(c + 1) * EPC]
        d_loc = dst[c * EPC:(c + 1) * EPC]
        cu = s_loc // CHUNK
        cv = d_loc // CHUNK
        key = (cu * 4 + cv).astype(np.uint8)
        order = np.argsort(key, kind="stable")
        key_sorted = key[order]
        counts = np.bincount(key, minlength=NSEG)
        cap = max(cap, int(np.ceil(counts.max() / P)) * P)
        shard_data.append((s_loc, d_loc, cu, cv, key, order, key_sorted, counts))

    colseg = cap // P
    for c in range(N_CORES):
        s_loc, d_loc, cu, cv, key, order, key_sorted, counts = shard_data[c]
        starts = np.zeros(NSEG, dtype=np.int64)
        starts[1:] = np.cumsum(counts)[:-1]
        r_in_seg = np.arange(EPC, dtype=np.int64) - starts[key_sorted]

        lu = (s_loc - cu * CHUNK).astype(np.int16)
        lv = (d_loc - cv * CHUNK).astype(np.int16)
        pu = np.zeros((NSEG, cap), dtype=np.int16)
        pv = np.zeros((NSEG, cap), dtype=np.int16)
        pu[key_sorted, r_in_seg] = lu[order]
        pv[key_sorted, r_in_seg] = lv[order]

        in_maps.append({
            "h": h,
            "su": _wrap16(pu, cap),
            "sv": _wrap16(pv, cap),
        })
        p = (r_in_seg % P).astype(np.int64)
        col = (key_sorted.astype(np.int64) * colseg
               + 8 * (r_in_seg // MAX_SUB) + (r_in_seg % MAX_SUB) // P)
        recon.append((order, p, col))
    return in_maps, recon, cap


def kernel(h, src, dst):
    in_maps, recon, cap = _prepare(h, src, dst)
    nc = _get_prog(cap)
    res = run_bass_kernel_spmd(nc, in_maps, list(range(N_CORES)))

    out = np.empty(N_EDGES, dtype=np.float32)
    for c in range(N_CORES):
        order, p, col = recon[c]
        scores = res.results[c]["scores"]          # [128, NSEG*colseg]
        shard = np.empty(EPC, dtype=np.float32)
        shard[order] = scores[p, col]
        out[c * EPC:(c + 1) * EPC] = shard
    return out
